# revision 1
# baseline (speedup 1.0000x reference)
"""Trainium2 Bass kernel for nn_CTR_Block_77077483094613 (gnn_message_passing).

Strategy (data-parallel over N across 8 cores, 4 samples per core):

Math simplifications applied on host (all exact, verified vs reference):
  * softmax(x1[u]-x2[v], axis=v) is independent of u (x1 cancels), so the
    attention tensor collapses to s2[n,c,v] = softmax(-x2[n,c,v]) and the
    attention einsum collapses to g[n,o,t] = sum_v s2[n,c(o),v]*x3[n,o,t,v]
    broadcast over u.  w1/b1 are unused.
  * A-mix branch re-parameterized: einsum(A, conv4(x)) == conv4(H) + rank-1
    bias, with H = einsum('uv,nctv->nctu', A, x) computed on host (linear
    input transform, im2col-style).  The rank-1 bias b4[o]*rowsum(A)[u] is
    folded in as a 65th input channel of H.
  * All BatchNorms folded into conv weights/biases on host.

Device pipeline per sample:
  conv2+softmax -> s2 ; build s2-scaled conv3 weights (gpsimd bcast + DVE mul)
  g via 25 psum-accumulated strided matmuls (v-slices of x), sample pairs
  row-tiled on the PE (K=64 halves) ; conv4 on H (K=65) ; yb = relu(y2+g)
  fused on evac into a t-padded buffer ; tcn = 9 shifted-tap matmuls +
  residual conv accumulated in one psum ; final relu(x*1+bias) on evac.
"""

import numpy as np

N, CIN, COUT, T, V = 32, 64, 128, 256, 25
IC = COUT // 4
EPS = 1e-5
NCORES = 8
NS = N // NCORES          # samples per core
TV = T * V                # 6400
TILE = 500                # free-dim tile: 20 t positions x 25 u
NTILES = TV // TILE       # 13 (wait: 6400/500 = 12.8) -- handled below
PAD = 4 * V               # 100

_CACHE = {}
TCN_FP8 = False  # fp8e4m3 DoubleRow temporal conv (2x PE) vs bf16


def _patch_tile_drain():
    """walrus in this container allows only 1 sync-wait per CTRL inst; split
    the TileContext end-of-kernel drain accordingly."""
    import concourse.tile as tile
    from concourse import mybir
    from concourse.vector_clock import ScopedClock

    if getattr(tile.TileContext, "_drain_split_patched", False):
        return

    def _drain_and_barrier(self, tick_clock, wait_clock):
        drain_inst = self.nc.sync.drain()
        wait_clock.add_sem_waits(
            drain_inst.ins, ScopedClock({None: tick_clock.global_clock})
        )
        si = drain_inst.ins.sync_info
        waits = list(si.on_wait or [])
        if len(waits) > 1:
            si.on_wait = waits[:1]
            for w in waits[1:]:
                d2 = self.nc.sync.drain()
                d2.ins.sync_info = mybir.SyncInfo(on_wait=[w], on_update=[])
        self.nc.all_engine_barrier()
        assert self.sems is not None
        popped = self.nc._tile_sem_poison_stack.pop()
        assert popped is self._sem_poison
        self.nc.clear_and_free_semaphores(list(self.sems.allocated().values()))
        self.nc.all_engine_barrier()

    tile.TileContext._drain_and_barrier = _drain_and_barrier
    tile.TileContext._drain_split_patched = True


def _split_multi_waits(nc, mybir):
    """walrus here allows only 1 sync-wait per instruction: hoist extra waits
    onto same-engine NoOps inserted just before the instruction."""
    k = 0
    for fn in nc.m.functions:
        for bb in fn.blocks:
            insts = bb.instructions
            i = 0
            while i < len(insts):
                ins = insts[i]
                si = ins.sync_info
                waits = list(si.on_wait) if si and si.on_wait else []
                if len(waits) > 1:
                    si.on_wait = waits[:1]
                    for w in waits[1:]:
                        nop = mybir.InstNoOp(
                            name=f"wsplit-{k}",
                            engine=ins.engine,
                            ins=[],
                            outs=[],
                            sync_info=mybir.SyncInfo(on_wait=[w], on_update=[]),
                        )
                        k += 1
                        insts.insert(i, nop)
                        i += 1
                i += 1


def _build_nc():
    from contextlib import ExitStack

    import bass_rust
    import concourse.bass as bass
    import concourse.tile as tile
    from concourse import mybir

    _patch_tile_drain()
    f32 = mybir.dt.float32
    f32r = mybir.dt.float32r
    bf16 = mybir.dt.bfloat16

    nc = bass.Bass()

    # ---- DRAM parameters (per-core shapes) ----
    d_x = nc.declare_dram_parameter("x", [NS, CIN, TV], bf16, isOutput=False)
    d_h = nc.declare_dram_parameter("h", [NS, CIN + 1, TV], bf16, isOutput=False)
    d_xm = nc.declare_dram_parameter("xm", [NS, CIN, V], f32, isOutput=False)
    d_w2t2 = nc.declare_dram_parameter("w2t2", [2 * CIN, 2 * IC], f32, isOutput=False)
    d_b2p = nc.declare_dram_parameter("b2p", [2 * IC, 1], f32, isOutput=False)
    d_w3t2 = nc.declare_dram_parameter("w3t2", [2 * CIN, COUT], f32, isOutput=False)
    d_gbias = nc.declare_dram_parameter("gbias", [COUT, 1], f32, isOutput=False)
    d_w4t = nc.declare_dram_parameter("w4t", [CIN + 1, COUT], bf16, isOutput=False)
    d_wrt2 = nc.declare_dram_parameter("wrt2", [2 * CIN, COUT], bf16, isOutput=False)
    f8 = mybir.dt.float8e4
    if TCN_FP8:
        d_wtt8 = nc.declare_dram_parameter("wtt8", [COUT, 8 * COUT], f8, isOutput=False)
        d_wt8l = nc.declare_dram_parameter("wt8l", [COUT, COUT], f8, isOutput=False)
    else:
        d_wtt = nc.declare_dram_parameter("wtt", [COUT, 9 * COUT], bf16, isOutput=False)
    d_bout = nc.declare_dram_parameter("bout", [COUT, 1], f32, isOutput=False)
    d_sel2 = nc.declare_dram_parameter("sel2", [2, COUT], bf16, isOutput=False)
    d_out = nc.declare_dram_parameter("out", [NS, COUT, TV], f32, isOutput=True)

    # tile widths: 12 x 500 + 1 x 400 = 6400
    widths = [TILE] * 12 + [400]
    offs = np.cumsum([0] + widths).tolist()

    with tile.TileContext(nc) as tc, ExitStack() as ctx:
        const = ctx.enter_context(tc.tile_pool(name="const", bufs=1))
        xpool = ctx.enter_context(tc.tile_pool(name="xpair", bufs=4))
        hpool = ctx.enter_context(tc.tile_pool(name="htile", bufs=3))
        ybpool = ctx.enter_context(tc.tile_pool(name="yb", bufs=2))
        spool = ctx.enter_context(tc.tile_pool(name="small", bufs=2))
        w3spool = ctx.enter_context(tc.tile_pool(name="w3s", bufs=4))
        opool = ctx.enter_context(tc.tile_pool(name="otile", bufs=4))
        pg = ctx.enter_context(tc.tile_pool(name="pg", bufs=2, space="PSUM"))
        py = ctx.enter_context(tc.tile_pool(name="py", bufs=3, space="PSUM"))
        po = ctx.enter_context(tc.tile_pool(name="po", bufs=3, space="PSUM"))

        # ---- load constants (tcn weights first: warmup depends on them) ----
        if TCN_FP8:
            wtt8 = const.tile([COUT, 8 * COUT], f8)
            nc.sync.dma_start(wtt8[:], d_wtt8[:])
            wt8l = const.tile([COUT, COUT], f8)
            nc.sync.dma_start(wt8l[:], d_wt8l[:])
        else:
            wtt = const.tile([COUT, 9 * COUT], bf16)
            nc.sync.dma_start(wtt[:], d_wtt[:])
        w2t2 = const.tile([2 * CIN, 2 * IC], f32)
        nc.sync.dma_start(w2t2[:], d_w2t2[:])
        b2p = const.tile([2 * IC, 1], f32)
        nc.sync.dma_start(b2p[:], d_b2p[:])
        w3t2 = const.tile([2 * CIN, COUT], f32)
        nc.sync.dma_start(w3t2[:], d_w3t2[:])
        gbias = const.tile([COUT, 1], f32)
        nc.sync.dma_start(gbias[:], d_gbias[:])
        w4t = const.tile([CIN + 1, COUT], bf16)
        nc.sync.dma_start(w4t[:], d_w4t[:])
        wrt2 = const.tile([2 * CIN, COUT], bf16)
        nc.sync.dma_start(wrt2[:], d_wrt2[:])
        bout = const.tile([COUT, 1], f32)
        nc.sync.dma_start(bout[:], d_bout[:])
        sel2 = const.tile([2, COUT], bf16)
        nc.sync.dma_start(sel2[:], d_sel2[:])
        zeros = const.tile([COUT, TILE], f32)
        nc.vector.memset(zeros[:], 0.0)

        # ---- PE warmup: keep the HAM clock-gate open while the first x/h
        # DMAs stream in (otherwise the first ~50us run at 1.2 GHz) ----
        for i in range(30):
            wps = pg.tile([COUT, 512], f32, tag="pg")
            if TCN_FP8:
                nc.tensor.matmul(wps[:], wtt8[:, 0:COUT], wtt8[:, (i % 3) * 128:(i % 3) * 128 + 512])
            else:
                nc.tensor.matmul(wps[:], wtt[:, 0:COUT], wtt[:, (i % 5) * 128:(i % 5) * 128 + 512])

        w3s_tiles = []
        for pair in range(NS // 2):
            na, nb = 2 * pair, 2 * pair + 1
            xm2 = spool.tile([2 * CIN, V], f32)
            nc.sync.dma_start(xm2[:], d_xm[na:na + 2])

            # ---- conv2 (both samples, block-diag weights) + softmax ----
            px2 = py.tile([2 * IC, V], f32, tag="pyt")
            nc.tensor.matmul(px2[:], w2t2[:], xm2[:])
            e2 = spool.tile([2 * IC, V], f32)
            # exp(-(w2@xm + b2)) = Exp(in*-1 + (-b2)); b2p holds -b2
            nc.scalar.activation(
                e2[:], px2[:], mybir.ActivationFunctionType.Exp,
                bias=b2p[:, 0:1], scale=-1.0,
            )
            ssum = spool.tile([2 * IC, 1], f32)
            nc.vector.tensor_reduce(
                ssum[:], e2[:], mybir.AxisListType.X, mybir.AluOpType.add
            )
            rinv = spool.tile([2 * IC, 1], f32)
            nc.vector.reciprocal(rinv[:], ssum[:])
            s2 = spool.tile([2 * IC, V], bf16)
            nc.vector.tensor_scalar_mul(s2[:], e2[:], rinv[:, 0:1])

            # ---- replicate s2 across partitions, build s2-scaled conv3 wts ----
            # s2rowpair[h, (c,v)] = s2 for sample half h; then one K=2 matmul
            # against a 0/1 selection matrix replicates to all 128 partitions.
            s2rowpair = spool.tile([2, IC * V], bf16)
            nc.sync.dma_start(s2rowpair[0:1, :], s2[0:IC, :])
            nc.sync.dma_start(s2rowpair[1:2, :], s2[IC:, :])
            s2rep = spool.tile([2 * CIN, IC * V], f32)
            for half in range(2):
                o0h, o1h = half * 400, (half + 1) * 400
                ps = pg.tile([2 * CIN, 400], f32, tag="pg")
                nc.tensor.matmul(ps[:], sel2[:], s2rowpair[:, o0h:o1h])
                nc.vector.tensor_copy(s2rep[:, o0h:o1h], ps[:])
            # W3S[p, (v, m, c)] = w3t2[p, (m, c)] * s2[c, v]
            w3s = w3spool.tile([2 * CIN, V * COUT], bf16)
            in0 = (
                w3t2[:]
                .rearrange("p (m c) -> p m c", m=4)
                .unsqueeze(1)
                .broadcast_to([2 * CIN, V, 4, IC])
            )
            in1 = (
                s2rep[:]
                .rearrange("p (c v) -> p v c", c=IC)
                .unsqueeze(2)
                .broadcast_to([2 * CIN, V, 4, IC])
            )
            nc.gpsimd.tensor_tensor(
                w3s[:].rearrange("p (v m c) -> p v m c", v=V, m=4),
                in0,
                in1,
                mybir.AluOpType.mult,
            )
            w3s_tiles.append(w3s)

        for pair in range(NS // 2):
            na, nb = 2 * pair, 2 * pair + 1
            w3s = w3s_tiles[pair]
            x2t = xpool.tile([2 * CIN, TV], bf16)
            nc.sync.dma_start(x2t[:], d_x[na:na + 2])

            # ---- g: 25 accumulated strided matmuls per sample (row-tiled) ----
            pga = pg.tile([COUT, T], f32, tag="pg")
            pgb = pg.tile([COUT, T], f32, tag="pg")
            for v in range(V):
                lhs_a = w3s[0:CIN, v * COUT:(v + 1) * COUT]
                lhs_b = w3s[CIN:, v * COUT:(v + 1) * COUT]
                rhs_a = x2t[0:CIN, v::25]
                rhs_b = x2t[CIN:, v::25]
                nc.tensor.matmul(pga[:], lhs_a, rhs_a, start=(v == 0), stop=(v == V - 1))
                nc.tensor.matmul(pgb[:], lhs_b, rhs_b, start=(v == 0), stop=(v == V - 1))
            g_a = spool.tile([COUT, T], f32)
            g_b = spool.tile([COUT, T], f32)
            gsc = 8.0 if TCN_FP8 else 1.0
            nc.scalar.activation(g_a[:], pga[:], mybir.ActivationFunctionType.Identity,
                                 bias=gbias[:, 0:1], scale=gsc)
            nc.scalar.activation(g_b[:], pgb[:], mybir.ActivationFunctionType.Identity,
                                 bias=gbias[:, 0:1], scale=gsc)

            yb_pair = {}
            for n, gq in ((na, g_a), (nb, g_b)):
                ht = hpool.tile([CIN + 1, TV], bf16, tag="ht")
                nc.sync.dma_start(ht[:], d_h[n])
                xrow = x2t[0:CIN, :] if n == na else x2t[CIN:, :]
                wrrow = wrt2[0:CIN, :] if n == na else wrt2[CIN:, :]
                if not TCN_FP8:
                    yb = ybpool.tile([COUT, TV + 2 * PAD], bf16)
                    nc.vector.memset(yb[:, 0:PAD], 0.0)
                    nc.vector.memset(yb[:, PAD + TV:], 0.0)
                    yb_pair[n] = yb
                    for j, (o0, w) in enumerate(zip(offs[:-1], widths)):
                        pyt = py.tile([COUT, TILE], f32, tag="pyt")
                        nc.tensor.matmul(pyt[:, 0:w], w4t[:], ht[:, o0:o0 + w])
                        t0, tw = o0 // V, w // V
                        gview = gq[:, t0:t0 + tw].unsqueeze(2).broadcast_to([COUT, tw, V])
                        dst = yb[:, PAD + o0:PAD + o0 + w]
                        nc.vector.scalar_tensor_tensor(
                            dst.rearrange("p (t v) -> p t v", v=V),
                            pyt[:, 0:w].rearrange("p (t v) -> p t v", v=V),
                            0.0, gview,
                            mybir.AluOpType.bypass, mybir.AluOpType.add,
                        )
                        nc.scalar.activation(dst, dst, mybir.ActivationFunctionType.Relu)
                    for j, (o0, w) in enumerate(zip(offs[:-1], widths)):
                        pot = po.tile([COUT, TILE], f32, tag="pot")
                        nc.tensor.matmul(pot[:, 0:w], wrrow, xrow[:, o0:o0 + w],
                                         start=True, stop=False)
                        for k in range(9):
                            nc.tensor.matmul(
                                pot[:, 0:w], wtt[:, k * COUT:(k + 1) * COUT],
                                yb[:, o0 + k * V:o0 + k * V + w],
                                start=False, stop=(k == 8))
                        ot = opool.tile([COUT, TILE], f32, tag="ot")
                        if j % 2 == 0:
                            nc.scalar.activation(
                                ot[:, 0:w], pot[:, 0:w],
                                mybir.ActivationFunctionType.Relu,
                                bias=bout[:, 0:1], scale=1.0)
                        else:
                            nc.vector.scalar_tensor_tensor(
                                ot[:, 0:w], pot[:, 0:w], bout[:, 0:1], zeros[:, 0:w],
                                mybir.AluOpType.add, mybir.AluOpType.max)
                        nc.sync.dma_start(d_out[n][:, o0:o0 + w], ot[:, 0:w])
                else:
                    # u padded to 32 so tap shifts are 16B-aligned (DoubleRow
                    # requirement); yb holds 8*yb in fp8e4m3, taps in pairs.
                    YW, PAD32 = 32, 4 * 32
                    yb = ybpool.tile([COUT, YW * (T + 8)], f8)
                    nc.vector.memset(yb[:], 0.0)
                    # conv4H still on 500-wide (t,u25) tiles
                    for j, (o0, w) in enumerate(zip(offs[:-1], widths)):
                        pyt = py.tile([COUT, TILE], f32, tag="pyt")
                        nc.tensor.matmul(pyt[:, 0:w], w4t[:], ht[:, o0:o0 + w])
                        t0, tw = o0 // V, w // V
                        gview = gq[:, t0:t0 + tw].unsqueeze(2).broadcast_to([COUT, tw, V])
                        dst = (
                            yb[:, PAD32 + t0 * YW: PAD32 + (t0 + tw) * YW]
                            .rearrange("p (t u) -> p t u", u=YW)[:, :, 0:V]
                        )
                        # 8*(y2 + g) in fp8
                        nc.vector.scalar_tensor_tensor(
                            dst,
                            pyt[:, 0:w].rearrange("p (t v) -> p t v", v=V),
                            8.0, gview,
                            mybir.AluOpType.mult, mybir.AluOpType.add,
                        )
                        nc.scalar.activation(dst, dst, mybir.ActivationFunctionType.Relu)
                    # tcn on 16-t tiles (512 u32-cols = one psum bank)
                    for j in range(T // 16):
                        t0 = 16 * j
                        pot = po.tile([COUT, 16 * YW], f32, tag="pot")
                        pview = pot[:].rearrange("p (t u) -> p t u", u=YW)[:, :, 0:V]
                        nc.tensor.matmul(
                            pview,
                            wrrow,
                            xrow[:, t0 * V:(t0 + 16) * V].rearrange(
                                "p (t v) -> p t v", v=V),
                            start=True, stop=False)
                        for p in range(4):
                            lhs = wtt8[:, p * 2 * COUT:(p + 1) * 2 * COUT].rearrange(
                                "p (j o) -> p j o", j=2)
                            # rhs: [K, 2, 512], tap shift = 32 fp8 elems (16B-aligned)
                            base = (t0 + 2 * p) * YW
                            rhs = yb[:, base: base + 2 * YW + 512].copy()
                            pdim = tuple(list(rhs.ap)[0])
                            rhs.ap = bass_rust.VecI64Pair([pdim, (YW, 2), (1, 512)])
                            nc.tensor.matmul(
                                pot[:],
                                lhs,
                                rhs,
                                start=False, stop=False,
                                perf_mode=mybir.MatmulPerfMode.DoubleRow)
                        nc.tensor.matmul(
                            pot[:], wt8l[:], yb[:, (t0 + 8) * YW:(t0 + 24) * YW],
                            start=False, stop=True)
                        ot = opool.tile([COUT, 16 * V], f32, tag="ot")
                        nc.scalar.activation(
                            ot[:], pview,
                            mybir.ActivationFunctionType.Relu,
                            bias=bout[:, 0:1], scale=1.0 / 256.0)
                        nc.sync.dma_start(
                            d_out[n][:, t0 * V:(t0 + 16) * V], ot[:])


    _split_multi_waits(nc, mybir)
    return nc


def _host_prep(inputs):
    x = np.ascontiguousarray(inputs["x"], dtype=np.float32)
    A = np.asarray(inputs["A"], dtype=np.float32)

    s1 = inputs["bn1_g"] / np.sqrt(inputs["bn1_v"] + EPS)
    t1 = inputs["bn1_b"] - inputs["bn1_m"] * s1
    s2n = inputs["bn2_g"] / np.sqrt(inputs["bn2_v"] + EPS)
    t2n = inputs["bn2_b"] - inputs["bn2_m"] * s2n
    sr = inputs["bnr_g"] / np.sqrt(inputs["bnr_v"] + EPS)
    tr = inputs["bnr_b"] - inputs["bnr_m"] * sr

    w2t = np.asarray(inputs["w2"], np.float32).T                     # [64, 32]
    w2t2 = np.zeros((2 * CIN, 2 * IC), np.float32)
    w2t2[0:CIN, 0:IC] = w2t
    w2t2[CIN:, IC:] = w2t
    b2p = np.concatenate([-inputs["b2"], -inputs["b2"]]).astype(np.float32)[:, None]

    w3p = (inputs["w3"] * s1[:, None]).astype(np.float32)            # [128, 64]
    w3t2 = np.concatenate([w3p.T, w3p.T], axis=0).astype(np.float32)  # [128, 128]
    gbias = (s1 * inputs["b3"] + t1).astype(np.float32)[:, None]

    w4p = (inputs["w4"] * s1[:, None]).astype(np.float32)
    w4t = np.zeros((CIN + 1, COUT), np.float32)
    w4t[0:CIN, :] = w4p.T
    w4t[CIN, :] = s1 * inputs["b4"]

    wrp = (inputs["wr"] * sr[:, None]).astype(np.float32)
    wrt2 = np.concatenate([wrp.T, wrp.T], axis=0).astype(np.float32)

    wtp = (inputs["wt"][..., 0] * s2n[:, None, None]).astype(np.float32)  # [128,128,9]
    wtt = np.concatenate([wtp[:, :, k].T for k in range(9)], axis=1)
    wtt = np.ascontiguousarray(wtt, np.float32)                       # [128, 9*128]
    # fp8 tap-pair packing: wtt8[i, (p, j, o)] = 32 * wtp[o, i, 2p+j]
    wtt8 = np.zeros((COUT, 8 * COUT), np.float32)
    for p in range(4):
        for j in range(2):
            wtt8[:, (2 * p + j) * COUT:(2 * p + j + 1) * COUT] = 32.0 * wtp[:, :, 2 * p + j].T
    wt8l = np.ascontiguousarray(32.0 * wtp[:, :, 8].T, np.float32)

    bout = (inputs["bt"] * s2n + t2n + inputs["br"] * sr + tr).astype(np.float32)[:, None]

    # H with rank-1 bias channel: h[n, 0:64, t, u] = sum_v A[u,v] x[n,:,t,v]
    # h[n, 64, t, u] = rowsum(A)[u]
    xf = x.reshape(N * CIN * T, V)
    H = (xf @ A.T).reshape(N, CIN, T, V)
    rA = A.sum(axis=1).astype(np.float32)
    h = np.empty((N, CIN + 1, T * V), np.float32)
    h[:, 0:CIN, :] = H.reshape(N, CIN, T * V)
    h[:, CIN, :] = np.tile(rA, T)[None, :]

    xm = x.mean(axis=2).astype(np.float32)                            # [N, 64, 25]

    sel2 = np.zeros((2, COUT), np.float32)
    sel2[0, 0:CIN] = 1.0
    sel2[1, CIN:] = 1.0

    import ml_dtypes
    bf = ml_dtypes.bfloat16
    f8 = ml_dtypes.float8_e4m3
    consts = dict(w2t2=w2t2, b2p=b2p, w3t2=w3t2,
                  w4t=w4t.astype(bf), bout=bout, sel2=sel2.astype(bf))
    if TCN_FP8:
        # yb carries 8*value; tcn weights carry 32x; residual conv carries
        # 256x so everything in the output psum shares one 256x scale.
        consts["gbias"] = 8.0 * gbias
        consts["wrt2"] = (256.0 * wrt2).astype(bf)
        consts["wtt8"] = wtt8.astype(f8)
        consts["wt8l"] = wt8l.astype(f8)
    else:
        consts["gbias"] = gbias
        consts["wrt2"] = wrt2.astype(bf)
        consts["wtt"] = wtt.astype(bf)
    return x.astype(bf), h.astype(bf), xm, consts


def kernel(**inputs):
    from concourse.bass_utils import run_bass_kernel_spmd

    x, h, xm, consts = _host_prep(inputs)

    if "nc" not in _CACHE:
        _CACHE["nc"] = _build_nc()
    nc = _CACHE["nc"]

    in_maps = []
    for core in range(NCORES):
        sl = slice(core * NS, (core + 1) * NS)
        m = dict(consts)
        m["x"] = np.ascontiguousarray(x[sl].reshape(NS, CIN, TV))
        m["h"] = np.ascontiguousarray(h[sl])
        m["xm"] = np.ascontiguousarray(xm[sl])
        in_maps.append(m)

    res = run_bass_kernel_spmd(nc, in_maps, list(range(NCORES)))
    out = np.concatenate([r["out"] for r in res.results], axis=0)
    return np.ascontiguousarray(out.reshape(N, COUT, T, V), dtype=np.float32)



# revision 4
# speedup vs baseline: 1.0482x; 1.0482x over previous
"""Trainium2 Bass kernel for nn_CTR_Block_77077483094613 (gnn_message_passing).

Strategy (data-parallel over N across 8 cores, 4 samples per core):

Math simplifications applied on host (all exact, verified vs reference):
  * softmax(x1[u]-x2[v], axis=v) is independent of u (x1 cancels), so the
    attention tensor collapses to s2[n,c,v] = softmax(-x2[n,c,v]) and the
    attention einsum collapses to g[n,o,t] = sum_v s2[n,c(o),v]*x3[n,o,t,v]
    broadcast over u.  w1/b1 are unused.
  * A-mix branch re-parameterized: einsum(A, conv4(x)) == conv4(H) + rank-1
    bias, with H = einsum('uv,nctv->nctu', A, x) computed on host (linear
    input transform, im2col-style).  The rank-1 bias b4[o]*rowsum(A)[u] is
    folded in as a 65th input channel of H.
  * All BatchNorms folded into conv weights/biases on host.

Device pipeline per sample:
  conv2+softmax -> s2 ; build s2-scaled conv3 weights (DVE/gpsimd bcast mul)
  g via 25 psum-accumulated strided matmuls (v-slices of x), sample pairs
  row-tiled on the PE (K=64 halves, dual-issued on disjoint row groups) ;
  conv4 on H (K=65) ; yb = relu(y2+g) fused on evac into a t-padded buffer ;
  tcn = 9 shifted-tap matmuls + residual conv accumulated in one psum ;
  final relu(x*1+bias) on evac.

Schedule notes (the perf-critical part):
  * PE warmup on a memset tile starts at ~t=0 (no DMA dependency) so the
    HAM clock-gate opens before real work arrives.
  * Bulk input DMAs (x pair0, h s0/s1, tcn weights) all emitted up-front on
    the Sync queue, BEFORE any output DMA, so outputs never head-of-line
    block inputs.  Pair-1 inputs (x, h s2/s3) prefetch on the GpSimd queue.
  * w3s for pair 0 is built on the DVE (split in two v-halves so the g
    matmuls can start after the first half); pair 1's w3s is built on the
    otherwise-idle GpSimd engine, off the critical path.
  * tcn residual convs for pair 1 are K=64 row-tiles at partitions 0/64 so
    the a/b sample pair dual-issues on disjoint PE row groups.
"""

import numpy as np

N, CIN, COUT, T, V = 32, 64, 128, 256, 25
IC = COUT // 4
EPS = 1e-5
NCORES = 8
NS = N // NCORES          # samples per core
TV = T * V                # 6400
TILE = 500                # free-dim tile: 20 t positions x 25 u
PAD = 4 * V               # 100

_CACHE = {}


def _patch_tile_drain():
    """walrus in this container allows only 1 sync-wait per CTRL inst; split
    the TileContext end-of-kernel drain accordingly."""
    import concourse.tile as tile
    from concourse import mybir
    from concourse.vector_clock import ScopedClock

    if getattr(tile.TileContext, "_drain_split_patched", False):
        return

    def _drain_and_barrier(self, tick_clock, wait_clock):
        drain_inst = self.nc.sync.drain()
        wait_clock.add_sem_waits(
            drain_inst.ins, ScopedClock({None: tick_clock.global_clock})
        )
        si = drain_inst.ins.sync_info
        waits = list(si.on_wait or [])
        if len(waits) > 1:
            si.on_wait = waits[:1]
            for w in waits[1:]:
                d2 = self.nc.sync.drain()
                d2.ins.sync_info = mybir.SyncInfo(on_wait=[w], on_update=[])
        self.nc.all_engine_barrier()
        assert self.sems is not None
        popped = self.nc._tile_sem_poison_stack.pop()
        assert popped is self._sem_poison
        self.nc.clear_and_free_semaphores(list(self.sems.allocated().values()))
        self.nc.all_engine_barrier()

    tile.TileContext._drain_and_barrier = _drain_and_barrier
    tile.TileContext._drain_split_patched = True


def _split_multi_waits(nc, mybir):
    """walrus here allows only 1 sync-wait per instruction: hoist extra waits
    onto same-engine NoOps inserted just before the instruction."""
    k = 0
    for fn in nc.m.functions:
        for bb in fn.blocks:
            insts = bb.instructions
            i = 0
            while i < len(insts):
                ins = insts[i]
                si = ins.sync_info
                waits = list(si.on_wait) if si and si.on_wait else []
                if len(waits) > 1:
                    si.on_wait = waits[:1]
                    for w in waits[1:]:
                        nop = mybir.InstNoOp(
                            name=f"wsplit-{k}",
                            engine=ins.engine,
                            ins=[],
                            outs=[],
                            sync_info=mybir.SyncInfo(on_wait=[w], on_update=[]),
                        )
                        k += 1
                        insts.insert(i, nop)
                        i += 1
                i += 1


def _build_nc():
    from contextlib import ExitStack

    import concourse.bass as bass
    import concourse.tile as tile
    from concourse import mybir

    _patch_tile_drain()
    f32 = mybir.dt.float32
    bf16 = mybir.dt.bfloat16

    nc = bass.Bass()

    # ---- DRAM parameters (per-core shapes) ----
    d_x = nc.declare_dram_parameter("x", [NS, CIN, TV], bf16, isOutput=False)
    d_h = nc.declare_dram_parameter("h", [NS, CIN + 1, TV], bf16, isOutput=False)
    d_xm = nc.declare_dram_parameter("xm", [NS, CIN, V], f32, isOutput=False)
    d_w2t2 = nc.declare_dram_parameter("w2t2", [2 * CIN, 2 * IC], f32, isOutput=False)
    d_b2p = nc.declare_dram_parameter("b2p", [2 * IC, 1], f32, isOutput=False)
    d_w3t2 = nc.declare_dram_parameter("w3t2", [2 * CIN, COUT], f32, isOutput=False)
    d_gbias = nc.declare_dram_parameter("gbias", [COUT, 1], f32, isOutput=False)
    d_w4t = nc.declare_dram_parameter("w4t", [CIN + 1, COUT], bf16, isOutput=False)
    d_wrt2 = nc.declare_dram_parameter("wrt2", [2 * CIN, COUT], bf16, isOutput=False)
    d_wtt = nc.declare_dram_parameter("wtt", [COUT, 9 * COUT], bf16, isOutput=False)
    d_bout = nc.declare_dram_parameter("bout", [COUT, 1], f32, isOutput=False)
    d_sel2 = nc.declare_dram_parameter("sel2", [2, COUT], bf16, isOutput=False)
    d_out = nc.declare_dram_parameter("out", [NS, COUT, TV], f32, isOutput=True)

    # tile widths: 12 x 500 + 1 x 400 = 6400
    widths = [TILE] * 12 + [400]
    offs = np.cumsum([0] + widths).tolist()

    with tile.TileContext(nc) as tc, ExitStack() as ctx:
        const = ctx.enter_context(tc.tile_pool(name="const", bufs=1))
        xpool = ctx.enter_context(tc.tile_pool(name="xpair", bufs=2))
        hpool = ctx.enter_context(tc.tile_pool(name="htile", bufs=3))
        ybpool = ctx.enter_context(tc.tile_pool(name="yb", bufs=3))
        spool = ctx.enter_context(tc.tile_pool(name="small", bufs=2))
        w3spool = ctx.enter_context(tc.tile_pool(name="w3s", bufs=2))
        opool = ctx.enter_context(tc.tile_pool(name="otile", bufs=6))
        pg = ctx.enter_context(tc.tile_pool(name="pg", bufs=2, space="PSUM"))
        py = ctx.enter_context(tc.tile_pool(name="py", bufs=2, space="PSUM"))
        po = ctx.enter_context(tc.tile_pool(name="po", bufs=4, space="PSUM"))

        # ---- tiny per-pair inputs on the GpSimd DMA queue (never blocked
        # behind bulk transfers) ----
        xm2s = []
        for pair in range(NS // 2):
            xm2 = spool.tile([2 * CIN, V], f32, tag=f"xm{pair}")
            nc.gpsimd.dma_start(xm2[:], d_xm[2 * pair:2 * pair + 2])
            xm2s.append(xm2)

        # ---- small consts on the Sync queue ----
        w2t2 = const.tile([2 * CIN, 2 * IC], f32)
        nc.sync.dma_start(w2t2[:], d_w2t2[:])
        b2p = const.tile([2 * IC, 1], f32)
        nc.sync.dma_start(b2p[:], d_b2p[:])
        sel2 = const.tile([2, COUT], bf16)
        nc.sync.dma_start(sel2[:], d_sel2[:])
        w3t2 = const.tile([2 * CIN, COUT], f32)
        nc.sync.dma_start(w3t2[:], d_w3t2[:])
        gbias = const.tile([COUT, 1], f32)
        nc.sync.dma_start(gbias[:], d_gbias[:])
        w4t = const.tile([CIN + 1, COUT], bf16)
        nc.sync.dma_start(w4t[:], d_w4t[:])
        wrt2 = const.tile([2 * CIN, COUT], bf16)
        nc.sync.dma_start(wrt2[:], d_wrt2[:])
        bout = const.tile([COUT, 1], f32)
        nc.sync.dma_start(bout[:], d_bout[:])

        # ---- PE warmup on a memset tile: opens the HAM clock-gate from
        # ~t=0 with no DMA dependency ----
        warm = const.tile([COUT, 512], bf16)
        nc.vector.memset(warm[:], 0.0)
        zeros = const.tile([COUT, TILE], f32)
        nc.vector.memset(zeros[:], 0.0)
        for i in range(10):
            wps = pg.tile([COUT, 512], f32, tag="pg")
            nc.tensor.matmul(wps[:], warm[:, 0:COUT], warm[:])

        # ---- bulk pair-0 inputs, in need-order, before any output DMA ----
        x2t_p0 = xpool.tile([2 * CIN, TV], bf16, tag="x2t")
        nc.sync.dma_start(x2t_p0[:], d_x[0:2])
        ht_s0 = hpool.tile([CIN + 1, TV], bf16, tag="ht")
        nc.sync.dma_start(ht_s0[:], d_h[0])
        ht_s1 = hpool.tile([CIN + 1, TV], bf16, tag="ht")
        nc.sync.dma_start(ht_s1[:], d_h[1])
        wtt = const.tile([COUT, 9 * COUT], bf16)
        nc.sync.dma_start(wtt[:], d_wtt[:])

        # ---- per-pair prologue: conv2 + softmax -> s2, replicate across
        # partitions, then build the s2-scaled conv3 weights ----
        def prologue(pair, w3s_engine):
            xm2 = xm2s[pair]
            px2 = py.tile([2 * IC, V], f32, tag="pyt")
            nc.tensor.matmul(px2[:], w2t2[:], xm2[:])
            e2 = spool.tile([2 * IC, V], f32, tag="e2")
            # exp(-(w2@xm + b2)) = Exp(in*-1 + (-b2)); b2p holds -b2
            nc.scalar.activation(
                e2[:], px2[:], mybir.ActivationFunctionType.Exp,
                bias=b2p[:, 0:1], scale=-1.0,
            )
            ssum = spool.tile([2 * IC, 1], f32, tag="ssum")
            nc.vector.tensor_reduce(
                ssum[:], e2[:], mybir.AxisListType.X, mybir.AluOpType.add
            )
            rinv = spool.tile([2 * IC, 1], f32, tag="rinv")
            nc.vector.reciprocal(rinv[:], ssum[:])
            s2 = spool.tile([2 * IC, V], bf16, tag="s2")
            nc.vector.tensor_scalar_mul(s2[:], e2[:], rinv[:, 0:1])

            # replicate s2 to all 128 partitions: pack both sample halves on
            # 2 partitions (GpSimd DMA queue), then one K=2 matmul against a
            # 0/1 selection matrix
            s2rowpair = spool.tile([2, IC * V], bf16, tag="s2row")
            nc.gpsimd.dma_start(s2rowpair[0:1, :], s2[0:IC, :])
            nc.gpsimd.dma_start(s2rowpair[1:2, :], s2[IC:, :])
            s2rep = spool.tile([2 * CIN, IC * V], f32, tag="s2rep")
            for half in range(2):
                o0h, o1h = half * 400, (half + 1) * 400
                ps = pg.tile([2 * CIN, 400], f32, tag="pg")
                nc.tensor.matmul(ps[:], sel2[:], s2rowpair[:, o0h:o1h])
                nc.vector.tensor_copy(s2rep[:, o0h:o1h], ps[:])

            # W3S[p, (v, m, c)] = w3t2[p, (m, c)] * s2[c, v]
            w3s = w3spool.tile([2 * CIN, V * COUT], bf16, tag="w3s")
            wv = w3s[:].rearrange("p (v m c) -> p v m c", v=V, m=4)
            i0 = w3t2[:].rearrange("p (m c) -> p m c", m=4).unsqueeze(1)
            i1 = s2rep[:].rearrange("p (c v) -> p v c", c=IC).unsqueeze(2)
            if w3s_engine == "vector":
                # split in two v-halves so the first g matmuls start early
                for v0, v1 in ((0, 12), (12, V)):
                    nc.vector.tensor_tensor(
                        wv[:, v0:v1],
                        i0.broadcast_to([2 * CIN, v1 - v0, 4, IC]),
                        i1[:, v0:v1].broadcast_to([2 * CIN, v1 - v0, 4, IC]),
                        mybir.AluOpType.mult,
                    )
            else:
                nc.gpsimd.tensor_tensor(
                    wv,
                    i0.broadcast_to([2 * CIN, V, 4, IC]),
                    i1.broadcast_to([2 * CIN, V, 4, IC]),
                    mybir.AluOpType.mult,
                )
            return w3s

        w3s_p0 = prologue(0, "vector")
        w3s_p1 = prologue(1, "gpsimd")

        # ---- pair-1 bulk inputs prefetch on the GpSimd queue ----
        x2t_p1 = xpool.tile([2 * CIN, TV], bf16, tag="x2t")
        nc.gpsimd.dma_start(x2t_p1[:], d_x[2:4])
        ht_s2 = hpool.tile([CIN + 1, TV], bf16, tag="ht")
        nc.gpsimd.dma_start(ht_s2[:], d_h[2])

        def g_pair(w3s, x2t):
            # g: 25 accumulated strided matmuls per sample, the two samples
            # row-tiled on disjoint PE row groups (dual-issue)
            pga = pg.tile([COUT, T], f32, tag="pg")
            pgb = pg.tile([COUT, T], f32, tag="pg")
            for v in range(V):
                nc.tensor.matmul(pga[:], w3s[0:CIN, v * COUT:(v + 1) * COUT],
                                 x2t[0:CIN, v::25], start=(v == 0), stop=(v == V - 1))
                nc.tensor.matmul(pgb[:], w3s[CIN:, v * COUT:(v + 1) * COUT],
                                 x2t[CIN:, v::25], start=(v == 0), stop=(v == V - 1))
            g_a = spool.tile([COUT, T], f32, tag="g_a")
            g_b = spool.tile([COUT, T], f32, tag="g_b")
            nc.scalar.activation(g_a[:], pga[:], mybir.ActivationFunctionType.Identity,
                                 bias=gbias[:, 0:1], scale=1.0)
            nc.scalar.activation(g_b[:], pgb[:], mybir.ActivationFunctionType.Identity,
                                 bias=gbias[:, 0:1], scale=1.0)
            return g_a, g_b

        def conv4(ht, gq):
            # yb = relu(conv4(H) + g), assembled into a t-padded buffer
            yb = ybpool.tile([COUT, TV + 2 * PAD], bf16, tag="yb")
            nc.vector.memset(yb[:, 0:PAD], 0.0)
            nc.vector.memset(yb[:, PAD + TV:], 0.0)
            for o0, w in zip(offs[:-1], widths):
                pyt = py.tile([COUT, TILE], f32, tag="pyt")
                nc.tensor.matmul(pyt[:, 0:w], w4t[:], ht[:, o0:o0 + w])
                t0, tw = o0 // V, w // V
                gview = gq[:, t0:t0 + tw].unsqueeze(2).broadcast_to([COUT, tw, V])
                dst = yb[:, PAD + o0:PAD + o0 + w]
                nc.vector.scalar_tensor_tensor(
                    dst.rearrange("p (t v) -> p t v", v=V),
                    pyt[:, 0:w].rearrange("p (t v) -> p t v", v=V),
                    0.0, gview,
                    mybir.AluOpType.bypass, mybir.AluOpType.add,
                )
                nc.scalar.activation(dst, dst, mybir.ActivationFunctionType.Relu)
            return yb

        def tcn_evac(pot, w, n, o0, j):
            # final relu(acc + bout), alternating engines to balance load
            ot = opool.tile([COUT, TILE], f32, tag="ot")
            if j % 2 == 0:
                nc.scalar.activation(
                    ot[:, 0:w], pot[:, 0:w],
                    mybir.ActivationFunctionType.Relu,
                    bias=bout[:, 0:1], scale=1.0)
            else:
                nc.vector.scalar_tensor_tensor(
                    ot[:, 0:w], pot[:, 0:w], bout[:, 0:1], zeros[:, 0:w],
                    mybir.AluOpType.add, mybir.AluOpType.max)
            nc.sync.dma_start(d_out[n][:, o0:o0 + w], ot[:, 0:w])

        def tcn_single(yb, xrow, wrrow, n):
            # tcn: 9 shifted-tap matmuls + residual conv in one psum
            for j, (o0, w) in enumerate(zip(offs[:-1], widths)):
                pot = po.tile([COUT, TILE], f32, tag="pot")
                nc.tensor.matmul(pot[:, 0:w], wrrow, xrow[:, o0:o0 + w],
                                 start=True, stop=False)
                for k in range(9):
                    nc.tensor.matmul(
                        pot[:, 0:w], wtt[:, k * COUT:(k + 1) * COUT],
                        yb[:, o0 + k * V:o0 + k * V + w],
                        start=False, stop=(k == 8))
                tcn_evac(pot, w, n, o0, j)

        def tcn_paired(yb_a, yb_b, x2t, na):
            # both samples per tile: the two K=64 residual convs dual-issue
            # on disjoint row groups; tap matmuls share weights
            for j, (o0, w) in enumerate(zip(offs[:-1], widths)):
                pot_a = po.tile([COUT, TILE], f32, tag="pot")
                pot_b = po.tile([COUT, TILE], f32, tag="pot")
                nc.tensor.matmul(pot_a[:, 0:w], wrt2[0:CIN, :], x2t[0:CIN, o0:o0 + w],
                                 start=True, stop=False)
                nc.tensor.matmul(pot_b[:, 0:w], wrt2[CIN:, :], x2t[CIN:, o0:o0 + w],
                                 start=True, stop=False)
                for k in range(9):
                    lhs = wtt[:, k * COUT:(k + 1) * COUT]
                    nc.tensor.matmul(pot_a[:, 0:w], lhs,
                                     yb_a[:, o0 + k * V:o0 + k * V + w],
                                     start=False, stop=(k == 8))
                    nc.tensor.matmul(pot_b[:, 0:w], lhs,
                                     yb_b[:, o0 + k * V:o0 + k * V + w],
                                     start=False, stop=(k == 8))
                tcn_evac(pot_a, w, na, o0, j)
                tcn_evac(pot_b, w, na + 1, o0, j + 1)

        # ---- pair 0: latency-optimized (per-sample tcn starts as soon as
        # the first conv4 evacs land) ----
        g_a0, g_b0 = g_pair(w3s_p0, x2t_p0)
        yb_s0 = conv4(ht_s0, g_a0)
        tcn_single(yb_s0, x2t_p0[0:CIN, :], wrt2[0:CIN, :], 0)
        yb_s1 = conv4(ht_s1, g_b0)
        tcn_single(yb_s1, x2t_p0[CIN:, :], wrt2[CIN:, :], 1)

        ht_s3 = hpool.tile([CIN + 1, TV], bf16, tag="ht")
        nc.gpsimd.dma_start(ht_s3[:], d_h[3])

        # ---- pair 1: throughput-optimized (paired tcn) ----
        g_a1, g_b1 = g_pair(w3s_p1, x2t_p1)
        yb_s2 = conv4(ht_s2, g_a1)
        yb_s3 = conv4(ht_s3, g_b1)
        tcn_paired(yb_s2, yb_s3, x2t_p1, 2)

    _split_multi_waits(nc, mybir)
    return nc


def _host_prep(inputs):
    x = np.ascontiguousarray(inputs["x"], dtype=np.float32)
    A = np.asarray(inputs["A"], dtype=np.float32)

    s1 = inputs["bn1_g"] / np.sqrt(inputs["bn1_v"] + EPS)
    t1 = inputs["bn1_b"] - inputs["bn1_m"] * s1
    s2n = inputs["bn2_g"] / np.sqrt(inputs["bn2_v"] + EPS)
    t2n = inputs["bn2_b"] - inputs["bn2_m"] * s2n
    sr = inputs["bnr_g"] / np.sqrt(inputs["bnr_v"] + EPS)
    tr = inputs["bnr_b"] - inputs["bnr_m"] * sr

    w2t = np.asarray(inputs["w2"], np.float32).T                     # [64, 32]
    w2t2 = np.zeros((2 * CIN, 2 * IC), np.float32)
    w2t2[0:CIN, 0:IC] = w2t
    w2t2[CIN:, IC:] = w2t
    b2p = np.concatenate([-inputs["b2"], -inputs["b2"]]).astype(np.float32)[:, None]

    w3p = (inputs["w3"] * s1[:, None]).astype(np.float32)            # [128, 64]
    w3t2 = np.concatenate([w3p.T, w3p.T], axis=0).astype(np.float32)  # [128, 128]
    gbias = (s1 * inputs["b3"] + t1).astype(np.float32)[:, None]

    w4p = (inputs["w4"] * s1[:, None]).astype(np.float32)
    w4t = np.zeros((CIN + 1, COUT), np.float32)
    w4t[0:CIN, :] = w4p.T
    w4t[CIN, :] = s1 * inputs["b4"]

    wrp = (inputs["wr"] * sr[:, None]).astype(np.float32)
    wrt2 = np.concatenate([wrp.T, wrp.T], axis=0).astype(np.float32)

    wtp = (inputs["wt"][..., 0] * s2n[:, None, None]).astype(np.float32)  # [128,128,9]
    wtt = np.concatenate([wtp[:, :, k].T for k in range(9)], axis=1)
    wtt = np.ascontiguousarray(wtt, np.float32)                       # [128, 9*128]

    bout = (inputs["bt"] * s2n + t2n + inputs["br"] * sr + tr).astype(np.float32)[:, None]

    # H with rank-1 bias channel: h[n, 0:64, t, u] = sum_v A[u,v] x[n,:,t,v]
    # h[n, 64, t, u] = rowsum(A)[u]
    xf = x.reshape(N * CIN * T, V)
    H = (xf @ A.T).reshape(N, CIN, T, V)
    rA = A.sum(axis=1).astype(np.float32)
    h = np.empty((N, CIN + 1, T * V), np.float32)
    h[:, 0:CIN, :] = H.reshape(N, CIN, T * V)
    h[:, CIN, :] = np.tile(rA, T)[None, :]

    xm = x.mean(axis=2).astype(np.float32)                            # [N, 64, 25]

    sel2 = np.zeros((2, COUT), np.float32)
    sel2[0, 0:CIN] = 1.0
    sel2[1, CIN:] = 1.0

    import ml_dtypes
    bf = ml_dtypes.bfloat16
    consts = dict(w2t2=w2t2, b2p=b2p, w3t2=w3t2, gbias=gbias,
                  w4t=w4t.astype(bf), bout=bout, sel2=sel2.astype(bf),
                  wrt2=wrt2.astype(bf), wtt=wtt.astype(bf))
    return x.astype(bf), h.astype(bf), xm, consts


def kernel(**inputs):
    from concourse.bass_utils import run_bass_kernel_spmd

    x, h, xm, consts = _host_prep(inputs)

    if "nc" not in _CACHE:
        _CACHE["nc"] = _build_nc()
    nc = _CACHE["nc"]

    in_maps = []
    for core in range(NCORES):
        sl = slice(core * NS, (core + 1) * NS)
        m = dict(consts)
        m["x"] = np.ascontiguousarray(x[sl].reshape(NS, CIN, TV))
        m["h"] = np.ascontiguousarray(h[sl])
        m["xm"] = np.ascontiguousarray(xm[sl])
        in_maps.append(m)

    res = run_bass_kernel_spmd(nc, in_maps, list(range(NCORES)))
    out = np.concatenate([r["out"] for r in res.results], axis=0)
    return np.ascontiguousarray(out.reshape(N, COUT, T, V), dtype=np.float32)


# revision 12
# speedup vs baseline: 1.0517x; 1.0033x over previous
"""Trainium2 Bass kernel for nn_CTR_Block_77077483094613 (gnn_message_passing).

Strategy (data-parallel over N across 8 cores, 4 samples per core):

Math simplifications applied on host (all exact, verified vs reference):
  * softmax(x1[u]-x2[v], axis=v) is independent of u (x1 cancels), so the
    attention tensor collapses to s2[n,c,v] = softmax(-x2[n,c,v]) and the
    attention einsum collapses to g[n,o,t] = sum_v s2[n,c(o),v]*x3[n,o,t,v]
    broadcast over u.  w1/b1 are unused.
  * A-mix branch re-parameterized: einsum(A, conv4(x)) == conv4(H) + rank-1
    bias, with H = einsum('uv,nctv->nctu', A, x) computed on host (linear
    input transform, im2col-style).  The rank-1 bias b4[o]*rowsum(A)[u] is
    folded in as a 65th input channel of H.
  * All BatchNorms folded into conv weights/biases on host.

Device pipeline per sample:
  conv2+softmax -> s2 ; build s2-scaled conv3 weights (DVE/gpsimd bcast mul)
  g via 25 psum-accumulated strided matmuls (v-slices of x), sample pairs
  row-tiled on the PE (K=64 halves, dual-issued on disjoint row groups) ;
  conv4 on H (K=65) ; yb = relu(y2+g) fused on evac into a t-padded buffer ;
  tcn = 9 shifted-tap matmuls + residual conv accumulated in one psum ;
  final relu(x*1+bias) on evac.

Schedule notes (the perf-critical part):
  * PE warmup on a memset tile starts at ~t=0 (no DMA dependency) so the
    HAM clock-gate opens before real work arrives.
  * Bulk input DMAs (x pair0, h s0/s1, tcn weights) all emitted up-front on
    the Sync queue, BEFORE any output DMA, so outputs never head-of-line
    block inputs.  Pair-1 inputs (x, h s2/s3) prefetch on the GpSimd queue.
  * w3s for pair 0 is built on the DVE (split in two v-halves so the g
    matmuls can start after the first half); pair 1's w3s is built on the
    otherwise-idle GpSimd engine, off the critical path.
  * tcn residual convs for pair 1 are K=64 row-tiles at partitions 0/64 so
    the a/b sample pair dual-issues on disjoint PE row groups.
"""

import numpy as np

N, CIN, COUT, T, V = 32, 64, 128, 256, 25
IC = COUT // 4
EPS = 1e-5
NCORES = 8
NS = N // NCORES          # samples per core
TV = T * V                # 6400
TILE = 500                # free-dim tile: 20 t positions x 25 u
PAD = 4 * V               # 100

_CACHE = {}


def _patch_tile_drain():
    """walrus in this container allows only 1 sync-wait per CTRL inst; split
    the TileContext end-of-kernel drain accordingly."""
    import concourse.tile as tile
    from concourse import mybir
    from concourse.vector_clock import ScopedClock

    if getattr(tile.TileContext, "_drain_split_patched", False):
        return

    def _drain_and_barrier(self, tick_clock, wait_clock):
        drain_inst = self.nc.sync.drain()
        wait_clock.add_sem_waits(
            drain_inst.ins, ScopedClock({None: tick_clock.global_clock})
        )
        si = drain_inst.ins.sync_info
        waits = list(si.on_wait or [])
        if len(waits) > 1:
            si.on_wait = waits[:1]
            for w in waits[1:]:
                d2 = self.nc.sync.drain()
                d2.ins.sync_info = mybir.SyncInfo(on_wait=[w], on_update=[])
        self.nc.all_engine_barrier()
        assert self.sems is not None
        popped = self.nc._tile_sem_poison_stack.pop()
        assert popped is self._sem_poison
        self.nc.clear_and_free_semaphores(list(self.sems.allocated().values()))
        self.nc.all_engine_barrier()

    tile.TileContext._drain_and_barrier = _drain_and_barrier
    tile.TileContext._drain_split_patched = True


def _split_multi_waits(nc, mybir):
    """walrus here allows only 1 sync-wait per instruction: hoist extra waits
    onto same-engine NoOps inserted just before the instruction."""
    k = 0
    for fn in nc.m.functions:
        for bb in fn.blocks:
            insts = bb.instructions
            i = 0
            while i < len(insts):
                ins = insts[i]
                si = ins.sync_info
                waits = list(si.on_wait) if si and si.on_wait else []
                if len(waits) > 1:
                    si.on_wait = waits[:1]
                    for w in waits[1:]:
                        nop = mybir.InstNoOp(
                            name=f"wsplit-{k}",
                            engine=ins.engine,
                            ins=[],
                            outs=[],
                            sync_info=mybir.SyncInfo(on_wait=[w], on_update=[]),
                        )
                        k += 1
                        insts.insert(i, nop)
                        i += 1
                i += 1


def _build_nc():
    from contextlib import ExitStack

    import concourse.bass as bass
    import concourse.tile as tile
    from concourse import mybir

    _patch_tile_drain()
    f32 = mybir.dt.float32
    bf16 = mybir.dt.bfloat16

    nc = bass.Bass()

    # ---- DRAM parameters (per-core shapes) ----
    # small consts packed into two blobs so the prologue pays 2 DMA-issue
    # latencies instead of 9 (each dma_start costs ~620ns on its engine)
    d_x = nc.declare_dram_parameter("x", [NS, CIN, TV], bf16, isOutput=False)
    d_h = nc.declare_dram_parameter("h", [NS, CIN + 1, TV], bf16, isOutput=False)
    d_xm = nc.declare_dram_parameter("xm", [2 * CIN, (NS // 2) * V], f32, isOutput=False)
    # cf32 cols: w2t2 0:64 | w3t2 64:192 | gbias 192 | bout 193 | b2p 194
    d_cf32 = nc.declare_dram_parameter("cf32", [2 * CIN, 195], f32, isOutput=False)
    # cb16 cols: w4t 0:128 | wrt2 128:256 | sel2 256:384
    d_cb16 = nc.declare_dram_parameter("cb16", [2 * CIN, 384], bf16, isOutput=False)
    d_wtt = nc.declare_dram_parameter("wtt", [COUT, 9 * COUT], bf16, isOutput=False)
    d_out = nc.declare_dram_parameter("out", [NS, COUT, TV], f32, isOutput=True)

    # tile widths: 12 x 500 + 1 x 400 = 6400
    widths = [TILE] * 12 + [400]
    offs = np.cumsum([0] + widths).tolist()

    with tile.TileContext(nc) as tc, ExitStack() as ctx:
        const = ctx.enter_context(tc.tile_pool(name="const", bufs=1))
        xpool = ctx.enter_context(tc.tile_pool(name="xpair", bufs=2))
        hpool = ctx.enter_context(tc.tile_pool(name="htile", bufs=3))
        ybpool = ctx.enter_context(tc.tile_pool(name="yb", bufs=3))
        spool = ctx.enter_context(tc.tile_pool(name="small", bufs=2))
        w3spool = ctx.enter_context(tc.tile_pool(name="w3s", bufs=2))
        opool = ctx.enter_context(tc.tile_pool(name="otile", bufs=6))
        pg = ctx.enter_context(tc.tile_pool(name="pg", bufs=2, space="PSUM"))
        py = ctx.enter_context(tc.tile_pool(name="py", bufs=2, space="PSUM"))
        po = ctx.enter_context(tc.tile_pool(name="po", bufs=4, space="PSUM"))

        # ---- tiny per-pair inputs on the GpSimd DMA queue (never blocked
        # behind bulk transfers); both pairs in one DMA ----
        xmall = spool.tile([2 * CIN, (NS // 2) * V], f32, tag="xm")
        nc.gpsimd.dma_start(xmall[:], d_xm[:])
        xm2s = [xmall[:, pair * V:(pair + 1) * V] for pair in range(NS // 2)]

        # ---- const blobs + bulk pair-0 inputs on the Sync queue, in
        # need-order, before any output DMA ----
        cf32 = const.tile([2 * CIN, 195], f32)
        nc.sync.dma_start(cf32[:], d_cf32[:])
        w2t2 = cf32[:, 0:2 * IC]
        w3t2 = cf32[:, 2 * IC:2 * IC + COUT]
        gbias = cf32[:, 192:193]
        bout = cf32[:, 193:194]
        b2p = cf32[0:2 * IC, 194:195]
        cb16 = const.tile([2 * CIN, 384], bf16)
        nc.sync.dma_start(cb16[:], d_cb16[:])
        w4t = cb16[0:CIN + 1, 0:COUT]
        wrt2a = cb16[0:CIN, COUT:2 * COUT]
        wrt2b = cb16[CIN:2 * CIN, COUT:2 * COUT]
        sel2 = cb16[0:2, 2 * COUT:3 * COUT]

        # ---- PE warmup on a memset tile: opens the HAM clock-gate from
        # ~t=0 with no DMA dependency ----
        warm = const.tile([COUT, 512], bf16)
        nc.vector.memset(warm[:], 0.0)
        zeros = const.tile([COUT, TILE], f32)
        nc.vector.memset(zeros[:], 0.0)
        for i in range(10):
            wps = pg.tile([COUT, 512], f32, tag="pg")
            nc.tensor.matmul(wps[:], warm[:, 0:COUT], warm[:])

        x2t_p0 = xpool.tile([2 * CIN, TV], bf16, tag="x2t")
        nc.sync.dma_start(x2t_p0[:], d_x[0:2])
        ht_s0 = hpool.tile([CIN + 1, TV], bf16, tag="ht")
        nc.sync.dma_start(ht_s0[:], d_h[0])
        wtt = const.tile([COUT, 9 * COUT], bf16)
        nc.sync.dma_start(wtt[:], d_wtt[:])
        ht_s1 = hpool.tile([CIN + 1, TV], bf16, tag="ht")
        nc.sync.dma_start(ht_s1[:], d_h[1])

        # ---- per-pair prologue: conv2 + softmax -> s2, replicate across
        # partitions, then build the s2-scaled conv3 weights ----
        def prologue(pair, w3s_engine):
            xm2 = xm2s[pair]
            px2 = py.tile([2 * IC, V], f32, tag="pyt")
            nc.tensor.matmul(px2[:], w2t2, xm2)
            e2 = spool.tile([2 * IC, V], f32, tag="e2")
            # exp(-(w2@xm + b2)) = Exp(in*-1 + (-b2)); b2p holds -b2
            nc.scalar.activation(
                e2[:], px2[:], mybir.ActivationFunctionType.Exp,
                bias=b2p, scale=-1.0,
            )
            ssum = spool.tile([2 * IC, 1], f32, tag="ssum")
            nc.vector.tensor_reduce(
                ssum[:], e2[:], mybir.AxisListType.X, mybir.AluOpType.add
            )
            rinv = spool.tile([2 * IC, 1], f32, tag="rinv")
            nc.vector.reciprocal(rinv[:], ssum[:])
            s2 = spool.tile([2 * IC, V], bf16, tag="s2")
            nc.vector.tensor_scalar_mul(s2[:], e2[:], rinv[:, 0:1])

            # replicate s2 to all 128 partitions: pack both sample halves on
            # 2 partitions (one partition-collapse DMA on the GpSimd queue),
            # then one K=2 matmul against a 0/1 selection matrix
            s2rowpair = spool.tile([2, IC * V], bf16, tag="s2row")
            nc.gpsimd.dma_start(s2rowpair[0:1, :], s2[0:IC, :])
            nc.gpsimd.dma_start(s2rowpair[1:2, :], s2[IC:, :])
            s2rep = spool.tile([2 * CIN, IC * V], f32, tag="s2rep")
            for half in range(2):
                o0h, o1h = half * 400, (half + 1) * 400
                ps = pg.tile([2 * CIN, 400], f32, tag="pg")
                nc.tensor.matmul(ps[:], sel2, s2rowpair[:, o0h:o1h])
                nc.vector.tensor_copy(s2rep[:, o0h:o1h], ps[:])

            # W3S[p, (v, m, c)] = w3t2[p, (m, c)] * s2[c, v]
            w3s = w3spool.tile([2 * CIN, V * COUT], bf16, tag="w3s")
            wv = w3s[:].rearrange("p (v m c) -> p v m c", v=V, m=4)
            i0 = w3t2.rearrange("p (m c) -> p m c", m=4).unsqueeze(1)
            i1 = s2rep[:].rearrange("p (c v) -> p v c", c=IC).unsqueeze(2)
            if w3s_engine == "split":
                # first v-half on DVE, second on GpSimd, in parallel; the g
                # matmuls consume w3s in v order so they start after half 1
                for eng, (v0, v1) in ((nc.vector, (0, 12)), (nc.gpsimd, (12, V))):
                    eng.tensor_tensor(
                        wv[:, v0:v1],
                        i0.broadcast_to([2 * CIN, v1 - v0, 4, IC]),
                        i1[:, v0:v1].broadcast_to([2 * CIN, v1 - v0, 4, IC]),
                        mybir.AluOpType.mult,
                    )
            else:
                nc.gpsimd.tensor_tensor(
                    wv,
                    i0.broadcast_to([2 * CIN, V, 4, IC]),
                    i1.broadcast_to([2 * CIN, V, 4, IC]),
                    mybir.AluOpType.mult,
                )
            return w3s

        w3s_p0 = prologue(0, "split")
        w3s_p1 = prologue(1, "gpsimd")

        # ---- pair-1 bulk inputs prefetch on the GpSimd queue ----
        x2t_p1 = xpool.tile([2 * CIN, TV], bf16, tag="x2t")
        nc.gpsimd.dma_start(x2t_p1[:], d_x[2:4])
        ht_s2 = hpool.tile([CIN + 1, TV], bf16, tag="ht")
        nc.gpsimd.dma_start(ht_s2[:], d_h[2])

        def g_pair(w3s, x2t):
            # g: 25 accumulated strided matmuls per sample, the two samples
            # row-tiled on disjoint PE row groups (dual-issue)
            pga = pg.tile([COUT, T], f32, tag="pg")
            pgb = pg.tile([COUT, T], f32, tag="pg")
            for v in range(V):
                nc.tensor.matmul(pga[:], w3s[0:CIN, v * COUT:(v + 1) * COUT],
                                 x2t[0:CIN, v::25], start=(v == 0), stop=(v == V - 1))
                nc.tensor.matmul(pgb[:], w3s[CIN:, v * COUT:(v + 1) * COUT],
                                 x2t[CIN:, v::25], start=(v == 0), stop=(v == V - 1))
            g_a = spool.tile([COUT, T], f32, tag="g_a")
            g_b = spool.tile([COUT, T], f32, tag="g_b")
            nc.scalar.activation(g_a[:], pga[:], mybir.ActivationFunctionType.Identity,
                                 bias=gbias, scale=1.0)
            nc.scalar.activation(g_b[:], pgb[:], mybir.ActivationFunctionType.Identity,
                                 bias=gbias, scale=1.0)
            return g_a, g_b

        def conv4(ht, gq):
            # yb = relu(conv4(H) + g), assembled into a t-padded buffer
            yb = ybpool.tile([COUT, TV + 2 * PAD], bf16, tag="yb")
            nc.vector.memset(yb[:, 0:PAD], 0.0)
            nc.vector.memset(yb[:, PAD + TV:], 0.0)
            for o0, w in zip(offs[:-1], widths):
                pyt = py.tile([COUT, TILE], f32, tag="pyt")
                nc.tensor.matmul(pyt[:, 0:w], w4t, ht[:, o0:o0 + w])
                t0, tw = o0 // V, w // V
                gview = gq[:, t0:t0 + tw].unsqueeze(2).broadcast_to([COUT, tw, V])
                dst = yb[:, PAD + o0:PAD + o0 + w]
                nc.vector.scalar_tensor_tensor(
                    dst.rearrange("p (t v) -> p t v", v=V),
                    pyt[:, 0:w].rearrange("p (t v) -> p t v", v=V),
                    0.0, gview,
                    mybir.AluOpType.bypass, mybir.AluOpType.add,
                )
                nc.scalar.activation(dst, dst, mybir.ActivationFunctionType.Relu)
            return yb

        def tcn_evac(pot, w, n, o0, j):
            # final relu(acc + bout), alternating engines to balance load
            ot = opool.tile([COUT, TILE], f32, tag="ot")
            if j % 2 == 0:
                nc.scalar.activation(
                    ot[:, 0:w], pot[:, 0:w],
                    mybir.ActivationFunctionType.Relu,
                    bias=bout, scale=1.0)
            else:
                nc.vector.scalar_tensor_tensor(
                    ot[:, 0:w], pot[:, 0:w], bout, zeros[:, 0:w],
                    mybir.AluOpType.add, mybir.AluOpType.max)
            nc.sync.dma_start(d_out[n][:, o0:o0 + w], ot[:, 0:w])

        def tcn_single(yb, xrow, wrrow, n):
            # tcn: 9 shifted-tap matmuls + residual conv in one psum
            for j, (o0, w) in enumerate(zip(offs[:-1], widths)):
                pot = po.tile([COUT, TILE], f32, tag="pot")
                nc.tensor.matmul(pot[:, 0:w], wrrow, xrow[:, o0:o0 + w],
                                 start=True, stop=False)
                for k in range(9):
                    nc.tensor.matmul(
                        pot[:, 0:w], wtt[:, k * COUT:(k + 1) * COUT],
                        yb[:, o0 + k * V:o0 + k * V + w],
                        start=False, stop=(k == 8))
                tcn_evac(pot, w, n, o0, j)

        def tcn_paired(yb_a, yb_b, x2t, na):
            # both samples per tile: the two K=64 residual convs dual-issue
            # on disjoint row groups; tap matmuls share weights
            for j, (o0, w) in enumerate(zip(offs[:-1], widths)):
                pot_a = po.tile([COUT, TILE], f32, tag="pot")
                pot_b = po.tile([COUT, TILE], f32, tag="pot")
                nc.tensor.matmul(pot_a[:, 0:w], wrt2a, x2t[0:CIN, o0:o0 + w],
                                 start=True, stop=False)
                nc.tensor.matmul(pot_b[:, 0:w], wrt2b, x2t[CIN:, o0:o0 + w],
                                 start=True, stop=False)
                for k in range(9):
                    lhs = wtt[:, k * COUT:(k + 1) * COUT]
                    nc.tensor.matmul(pot_a[:, 0:w], lhs,
                                     yb_a[:, o0 + k * V:o0 + k * V + w],
                                     start=False, stop=(k == 8))
                    nc.tensor.matmul(pot_b[:, 0:w], lhs,
                                     yb_b[:, o0 + k * V:o0 + k * V + w],
                                     start=False, stop=(k == 8))
                tcn_evac(pot_a, w, na, o0, j)
                tcn_evac(pot_b, w, na + 1, o0, j + 1)

        # ---- pair 0: latency-optimized (per-sample tcn starts as soon as
        # the first conv4 evacs land) ----
        g_a0, g_b0 = g_pair(w3s_p0, x2t_p0)
        yb_s0 = conv4(ht_s0, g_a0)
        tcn_single(yb_s0, x2t_p0[0:CIN, :], wrt2a, 0)
        yb_s1 = conv4(ht_s1, g_b0)
        tcn_single(yb_s1, x2t_p0[CIN:, :], wrt2b, 1)

        ht_s3 = hpool.tile([CIN + 1, TV], bf16, tag="ht")
        nc.gpsimd.dma_start(ht_s3[:], d_h[3])

        # ---- pair 1: throughput-optimized (paired tcn) ----
        g_a1, g_b1 = g_pair(w3s_p1, x2t_p1)
        yb_s2 = conv4(ht_s2, g_a1)
        yb_s3 = conv4(ht_s3, g_b1)
        tcn_paired(yb_s2, yb_s3, x2t_p1, 2)

    _split_multi_waits(nc, mybir)
    return nc


def _host_prep(inputs):
    x = np.ascontiguousarray(inputs["x"], dtype=np.float32)
    A = np.asarray(inputs["A"], dtype=np.float32)

    s1 = inputs["bn1_g"] / np.sqrt(inputs["bn1_v"] + EPS)
    t1 = inputs["bn1_b"] - inputs["bn1_m"] * s1
    s2n = inputs["bn2_g"] / np.sqrt(inputs["bn2_v"] + EPS)
    t2n = inputs["bn2_b"] - inputs["bn2_m"] * s2n
    sr = inputs["bnr_g"] / np.sqrt(inputs["bnr_v"] + EPS)
    tr = inputs["bnr_b"] - inputs["bnr_m"] * sr

    w2t = np.asarray(inputs["w2"], np.float32).T                     # [64, 32]
    w2t2 = np.zeros((2 * CIN, 2 * IC), np.float32)
    w2t2[0:CIN, 0:IC] = w2t
    w2t2[CIN:, IC:] = w2t
    b2p = np.concatenate([-inputs["b2"], -inputs["b2"]]).astype(np.float32)[:, None]

    w3p = (inputs["w3"] * s1[:, None]).astype(np.float32)            # [128, 64]
    w3t2 = np.concatenate([w3p.T, w3p.T], axis=0).astype(np.float32)  # [128, 128]
    gbias = (s1 * inputs["b3"] + t1).astype(np.float32)[:, None]

    w4p = (inputs["w4"] * s1[:, None]).astype(np.float32)
    w4t = np.zeros((CIN + 1, COUT), np.float32)
    w4t[0:CIN, :] = w4p.T
    w4t[CIN, :] = s1 * inputs["b4"]

    wrp = (inputs["wr"] * sr[:, None]).astype(np.float32)
    wrt2 = np.concatenate([wrp.T, wrp.T], axis=0).astype(np.float32)

    wtp = (inputs["wt"][..., 0] * s2n[:, None, None]).astype(np.float32)  # [128,128,9]
    wtt = np.concatenate([wtp[:, :, k].T for k in range(9)], axis=1)
    wtt = np.ascontiguousarray(wtt, np.float32)                       # [128, 9*128]

    bout = (inputs["bt"] * s2n + t2n + inputs["br"] * sr + tr).astype(np.float32)[:, None]

    # H with rank-1 bias channel: h[n, 0:64, t, u] = sum_v A[u,v] x[n,:,t,v]
    # h[n, 64, t, u] = rowsum(A)[u]
    xf = x.reshape(N * CIN * T, V)
    H = (xf @ A.T).reshape(N, CIN, T, V)
    rA = A.sum(axis=1).astype(np.float32)
    h = np.empty((N, CIN + 1, T * V), np.float32)
    h[:, 0:CIN, :] = H.reshape(N, CIN, T * V)
    h[:, CIN, :] = np.tile(rA, T)[None, :]

    xm = x.mean(axis=2).astype(np.float32)                            # [N, 64, 25]

    sel2 = np.zeros((2, COUT), np.float32)
    sel2[0, 0:CIN] = 1.0
    sel2[1, CIN:] = 1.0

    import ml_dtypes
    bf = ml_dtypes.bfloat16

    # pack small consts into two blobs (one DMA issue each on device)
    cf32 = np.zeros((2 * CIN, 195), np.float32)
    cf32[:, 0:2 * IC] = w2t2
    cf32[:, 2 * IC:2 * IC + COUT] = w3t2
    cf32[:, 192:193] = gbias
    cf32[:, 193:194] = bout
    cf32[0:2 * IC, 194:195] = b2p
    cb16 = np.zeros((2 * CIN, 384), np.float32)
    cb16[0:CIN + 1, 0:COUT] = w4t
    cb16[:, COUT:2 * COUT] = wrt2
    cb16[0:2, 2 * COUT:3 * COUT] = sel2

    consts = dict(cf32=cf32, cb16=cb16.astype(bf), wtt=wtt.astype(bf))
    return x.astype(bf), h.astype(bf), xm, consts


def kernel(**inputs):
    from concourse.bass_utils import run_bass_kernel_spmd

    x, h, xm, consts = _host_prep(inputs)

    if "nc" not in _CACHE:
        _CACHE["nc"] = _build_nc()
    nc = _CACHE["nc"]

    in_maps = []
    for core in range(NCORES):
        sl = slice(core * NS, (core + 1) * NS)
        m = dict(consts)
        m["x"] = np.ascontiguousarray(x[sl].reshape(NS, CIN, TV))
        m["h"] = np.ascontiguousarray(h[sl])
        xmc = xm[sl]                                   # [NS, CIN, V]
        xmblob = np.concatenate(
            [xmc.reshape(NS // 2, 2 * CIN, V)[p] for p in range(NS // 2)],
            axis=1,
        )                                              # [2*CIN, (NS//2)*V]
        m["xm"] = np.ascontiguousarray(xmblob)
        in_maps.append(m)

    res = run_bass_kernel_spmd(nc, in_maps, list(range(NCORES)))
    out = np.concatenate([r["out"] for r in res.results], axis=0)
    return np.ascontiguousarray(out.reshape(N, COUT, T, V), dtype=np.float32)
